# revision 14
# baseline (speedup 1.0000x reference)
"""Trainium2 Bass kernel for nn_Denoising_ResNet: out = x + conv1x1(box_mean3x3(x)) + b.

Sharding: data-parallel over batch (32 samples -> 4 per core x 8 cores).

Per-core layout: 2 "stacks" of 2 samples each -> 128 SBUF partitions
(= 2 samples x 64 channels). The image is processed in H-chunks of 32
rows with a 1-row halo.

Math decomposition (K=3 edge-clipped box mean, then 1x1 conv):
  - W-direction 3-tap sum: two shifted vector adds (fp32 in, bf16 out).
  - Global /9 of the box mean is folded into the conv weight.
  - Edge-clip count fix: boundary cols of the W-sum are scaled by 3/2
    (fold of 1/2 vs 1/3 count), boundary rows are fixed post-matmul on
    PSUM by 3/2. Corners get both (9/4 total vs folded 1/9 -> 1/4).
  - H-direction 3-tap sum is FUSED into the 1x1 conv: 3 accumulating
    PE matmuls with row-shifted moving operands against a block-diagonal
    [128,128] weight kron(I2, (W/9)^T) in bf16. Zeroed halo rows at the
    image boundary make the clipped sum come out naturally.
  - Final combine is one fused scalar_tensor_tensor:
      out = (psum + bias[p]) + x.
"""

from contextlib import ExitStack

import numpy as np

import concourse.bass as bass
import concourse.tile as tile
from concourse import bacc, mybir
from concourse.bass_utils import run_bass_kernel_spmd

B, C, H, W = 32, 64, 128, 128
NCORES = 8
PER = B // NCORES  # samples per core
NSTACK = PER // 2  # 2-sample stacks per core
HC = 32  # chunk height (output rows per chunk)
NCHUNK = H // HC
PIECE_ROWS = 4  # output rows per PSUM piece (4*128 = 512 f32 = 1 bank)

F32 = mybir.dt.float32
BF16 = mybir.dt.bfloat16


def _build_nc() -> bass.Bass:
    nc = bacc.Bacc("TRN2", debug=False)
    x = nc.dram_tensor("x", [PER * C, H, W], F32, kind="ExternalInput")
    w9t = nc.dram_tensor("w9t", [2 * C, 2 * C], BF16, kind="ExternalInput")
    bias2 = nc.dram_tensor("bias2", [2 * C, 1], F32, kind="ExternalInput")
    y = nc.dram_tensor("y", [PER * C, H, W], F32, kind="ExternalOutput")
    xap = x.ap()
    yap = y.ap()
    ADD = mybir.AluOpType.add

    with ExitStack() as ctx:
        tc = ctx.enter_context(tile.TileContext(nc))
        cpool = ctx.enter_context(tc.tile_pool(name="const", bufs=1))
        wt = cpool.tile([128, 128], BF16)
        nc.sync.dma_start(out=wt[:], in_=w9t.ap()[:, :])
        bt = cpool.tile([128, 1], F32)
        nc.sync.dma_start(out=bt[:], in_=bias2.ap()[:, :])

        # Warm-up: make PE observe wt's DMA and DVE observe bt's DMA once,
        # so the hot-loop instructions each carry a single sem wait
        # (walrus allows only one sync-wait per compute instruction).
        scratch = cpool.tile([128, 1], F32)
        ppool = ctx.enter_context(tc.tile_pool(name="psum", bufs=8, space="PSUM"))
        wps = ppool.tile([128, 1], F32, tag="ps")
        nc.tensor.matmul(wps[:], wt[:], wt[:, 0:1], start=True, stop=True)
        nc.vector.tensor_copy(scratch[:], bt[:])

        # bufs=4 on the DMA-touched pools: 2 sync-DMAs per chunk iteration
        # and 8 round-robin HWDGE queues means a reuse distance of 4 iters
        # puts the slot-reuse WAW dep on the SAME queue -> one sem wait.
        xpool = ctx.enter_context(tc.tile_pool(name="xin", bufs=4))
        tpool = ctx.enter_context(tc.tile_pool(name="tmp", bufs=2))
        wpool = ctx.enter_context(tc.tile_pool(name="wsum", bufs=2))
        opool = ctx.enter_context(tc.tile_pool(name="out", bufs=4))

        for g in range(NSTACK):
            p0 = g * 128
            for ci in range(NCHUNK):
                h0 = ci * HC
                # xt row r holds image row h0-1+r. Rows [ra, rb) hold real
                # image data; the halo row outside the image is never read
                # from xt — the corresponding ws row is zeroed instead (its
                # consumer is the PE matmul, whose DVE wait covers both).
                xt = xpool.tile([128, HC + 2, W], F32)
                ra, rb = 0, HC + 2
                if ci == 0:
                    ra = 1
                    nc.gpsimd.dma_start(
                        out=xt[:, 1 : HC + 2, :], in_=xap[p0 : p0 + 128, 0 : HC + 1, :]
                    )
                elif ci == NCHUNK - 1:
                    rb = HC + 1
                    nc.gpsimd.dma_start(
                        out=xt[:, 0 : HC + 1, :], in_=xap[p0 : p0 + 128, h0 - 1 : H, :]
                    )
                else:
                    nc.gpsimd.dma_start(
                        out=xt[:], in_=xap[p0 : p0 + 128, h0 - 1 : h0 + HC + 1, :]
                    )

                # W-direction 3-tap, edge cols folded to 1.5*(2-tap).
                tt = tpool.tile([128, HC + 2, W], F32)
                ws = wpool.tile([128, HC + 2, W], BF16)
                if ci == 0:
                    nc.vector.memset(ws[:, 0:1, :], 0.0)
                elif ci == NCHUNK - 1:
                    nc.vector.memset(ws[:, HC + 1 : HC + 2, :], 0.0)
                nc.vector.tensor_add(
                    tt[:, ra:rb, 1:W], xt[:, ra:rb, 0 : W - 1], xt[:, ra:rb, 1:W]
                )
                nc.vector.tensor_add(
                    ws[:, ra:rb, 1 : W - 1], tt[:, ra:rb, 1 : W - 1], xt[:, ra:rb, 2:W]
                )
                nc.vector.tensor_scalar_mul(ws[:, ra:rb, 0:1], tt[:, ra:rb, 1:2], 1.5)
                nc.vector.tensor_scalar_mul(
                    ws[:, ra:rb, W - 1 : W], tt[:, ra:rb, W - 1 : W], 1.5
                )

                ot = opool.tile([128, HC, W], F32)
                for p in range(HC // PIECE_ROWS):
                    r0 = p * PIECE_ROWS
                    ps = ppool.tile([128, PIECE_ROWS, W], F32, tag="ps")
                    for j, dh in enumerate((-1, 0, 1)):
                        rr = 1 + r0 + dh
                        nc.tensor.matmul(
                            ps[:],
                            wt[:],
                            ws[:, rr : rr + PIECE_ROWS, :],
                            start=(j == 0),
                            stop=(j == 2),
                        )
                    # image-boundary row needs the H-count fix (x1.5 on the
                    # conv term): emit it as (ps*1.5)+x then +bias, so no op
                    # needs more than one cross-engine sem wait.
                    fix_row = None
                    if ci == 0 and p == 0:
                        fix_row = 0
                    elif ci == NCHUNK - 1 and p == HC // PIECE_ROWS - 1:
                        fix_row = PIECE_ROWS - 1
                    if fix_row is None:
                        na, nb = 0, PIECE_ROWS
                    else:
                        na, nb = (1, PIECE_ROWS) if fix_row == 0 else (0, PIECE_ROWS - 1)
                        nc.vector.scalar_tensor_tensor(
                            ot[:, r0 + fix_row : r0 + fix_row + 1, :],
                            ps[:, fix_row : fix_row + 1, :],
                            1.5,
                            xt[:, 1 + r0 + fix_row : 2 + r0 + fix_row, :],
                            op0=mybir.AluOpType.mult,
                            op1=ADD,
                        )
                        nc.vector.tensor_scalar_add(
                            ot[:, r0 + fix_row : r0 + fix_row + 1, :],
                            ot[:, r0 + fix_row : r0 + fix_row + 1, :],
                            bt[:],
                        )
                    nc.vector.scalar_tensor_tensor(
                        ot[:, r0 + na : r0 + nb, :],
                        ps[:, na:nb, :],
                        bt[:],
                        xt[:, 1 + r0 + na : 1 + r0 + nb, :],
                        op0=ADD,
                        op1=ADD,
                    )
                nc.gpsimd.dma_start(out=yap[p0 : p0 + 128, h0 : h0 + HC, :], in_=ot[:])
    nc.compile()
    return nc


_NC = None


def _get_nc() -> bass.Bass:
    global _NC
    if _NC is None:
        _NC = _build_nc()
    return _NC


def _host_inputs(x: np.ndarray, conv_w: np.ndarray, conv_b: np.ndarray):
    import ml_dtypes

    w9t = np.zeros((2 * C, 2 * C), dtype=np.float32)
    wT = (conv_w.astype(np.float32) / 9.0).T
    w9t[0:C, 0:C] = wT
    w9t[C : 2 * C, C : 2 * C] = wT
    w9t = w9t.astype(ml_dtypes.bfloat16)
    bias2 = np.concatenate([conv_b, conv_b]).reshape(2 * C, 1).astype(np.float32)
    x = np.ascontiguousarray(x, dtype=np.float32)
    in_maps = []
    for i in range(NCORES):
        xi = x[i * PER : (i + 1) * PER].reshape(PER * C, H, W)
        in_maps.append({"x": xi, "w9t": w9t, "bias2": bias2})
    return in_maps


def kernel(x: np.ndarray, conv_w: np.ndarray, conv_b: np.ndarray) -> np.ndarray:
    nc = _get_nc()
    in_maps = _host_inputs(x, conv_w, conv_b)
    res = run_bass_kernel_spmd(nc, in_maps, list(range(NCORES)))
    outs = [
        np.asarray(res.results[i]["y"]).reshape(PER, C, H, W) for i in range(NCORES)
    ]
    return np.concatenate(outs, axis=0)


# revision 16
# speedup vs baseline: 1.2369x; 1.2369x over previous
"""Trainium2 Bass kernel for nn_Denoising_ResNet: out = x + conv1x1(box_mean3x3(x)) + b.

Sharding: data-parallel over batch (32 samples -> 4 per core x 8 cores).

Per-core layout: 2 "stacks" of 2 samples each -> 128 SBUF partitions
(= 2 samples x 64 channels). The image is processed in H-chunks of 32
rows with a 1-row halo.

Math decomposition (K=3 edge-clipped box mean, then 1x1 conv):
  - x is loaded as bf16 (gpsimd casting DMA).
  - W-direction 3-tap sum: two shifted adds (one on GpSimd, one on DVE),
    edge cols folded to 1.5*(2-tap) on DVE.
  - Global /9 of the box mean is folded into the conv weight.
  - H-direction 3-tap sum is FUSED into the 1x1 conv: 3 accumulating
    PE matmuls with row-shifted moving operands against a block-diagonal
    [128,128] weight kron(I2, (W/9)^T) in bf16.
  - The residual +x rides the same PSUM group as a 4th accumulating
    identity matmul (kron(I2, I) bf16).
  - PSUM -> SBUF copy + bias live on the scalar engine
    (activation Identity, bias per partition).
  - Image-boundary rows use their own PSUM group: 2 clipped H-matmuls +
    identity/1.5 for x, then the scalar-engine copy applies scale=1.5
    (the edge-clip row count fix) so only the conv term is scaled 1.5x
    while x comes out unscaled.
"""

from contextlib import ExitStack

import numpy as np

import concourse.bass as bass
import concourse.tile as tile
from concourse import bacc, mybir
from concourse.bass_utils import run_bass_kernel_spmd

B, C, H, W = 32, 64, 128, 128
NCORES = 8
PER = B // NCORES  # samples per core
NSTACK = PER // 2  # 2-sample stacks per core
HC = 32  # chunk height (output rows per chunk)
NCHUNK = H // HC
PIECE_ROWS = 4  # output rows per PSUM piece (4*128 = 512 f32 = 1 bank)
NPIECE = HC // PIECE_ROWS

F32 = mybir.dt.float32
BF16 = mybir.dt.bfloat16


def _build_nc() -> bass.Bass:
    nc = bacc.Bacc("TRN2", debug=False)
    x = nc.dram_tensor("x", [PER * C, H, W], F32, kind="ExternalInput")
    w9t = nc.dram_tensor("w9t", [2 * C, 2 * C], BF16, kind="ExternalInput")
    ident = nc.dram_tensor("ident", [2 * C, 2 * C], BF16, kind="ExternalInput")
    ident15 = nc.dram_tensor("ident15", [2 * C, 2 * C], BF16, kind="ExternalInput")
    bias2 = nc.dram_tensor("bias2", [2 * C, 1], F32, kind="ExternalInput")
    y = nc.dram_tensor("y", [PER * C, H, W], F32, kind="ExternalOutput")
    xap = x.ap()
    yap = y.ap()
    IDENT_FN = mybir.ActivationFunctionType.Identity

    with ExitStack() as ctx:
        tc = ctx.enter_context(tile.TileContext(nc))
        cpool = ctx.enter_context(tc.tile_pool(name="const", bufs=1))
        wt = cpool.tile([128, 128], BF16)
        nc.sync.dma_start(out=wt[:], in_=w9t.ap()[:, :])
        it = cpool.tile([128, 128], BF16)
        nc.sync.dma_start(out=it[:], in_=ident.ap()[:, :])
        it15 = cpool.tile([128, 128], BF16)
        nc.sync.dma_start(out=it15[:], in_=ident15.ap()[:, :])
        bt = cpool.tile([128, 1], F32)
        nc.sync.dma_start(out=bt[:], in_=bias2.ap()[:, :])

        ppool = ctx.enter_context(tc.tile_pool(name="psum", bufs=8, space="PSUM"))

        xpool = ctx.enter_context(tc.tile_pool(name="xin", bufs=4))
        tpool = ctx.enter_context(tc.tile_pool(name="tmp", bufs=2))
        wpool = ctx.enter_context(tc.tile_pool(name="wsum", bufs=2))
        opool = ctx.enter_context(tc.tile_pool(name="out", bufs=4))

        for g in range(NSTACK):
            p0 = g * 128
            for ci in range(NCHUNK):
                h0 = ci * HC
                # xt row r holds image row h0-1+r (bf16). Rows [ra, rb) are
                # real image data; out-of-image halo rows are never read.
                xt = xpool.tile([128, HC + 2, W], BF16)
                ra, rb = 0, HC + 2
                if ci == 0:
                    ra = 1
                    nc.gpsimd.dma_start(
                        out=xt[:, 1 : HC + 2, :], in_=xap[p0 : p0 + 128, 0 : HC + 1, :]
                    )
                elif ci == NCHUNK - 1:
                    rb = HC + 1
                    nc.gpsimd.dma_start(
                        out=xt[:, 0 : HC + 1, :], in_=xap[p0 : p0 + 128, h0 - 1 : H, :]
                    )
                else:
                    nc.gpsimd.dma_start(
                        out=xt[:], in_=xap[p0 : p0 + 128, h0 - 1 : h0 + HC + 1, :]
                    )

                # W-direction 3-tap: t = pairsum on GpSimd, finish on DVE.
                tt = tpool.tile([128, HC + 2, W], BF16)
                ws = wpool.tile([128, HC + 2, W], BF16)
                nc.vector.tensor_add(
                    tt[:, ra:rb, 1:W], xt[:, ra:rb, 0 : W - 1], xt[:, ra:rb, 1:W]
                )
                nc.vector.tensor_add(
                    ws[:, ra:rb, 1 : W - 1], tt[:, ra:rb, 1 : W - 1], xt[:, ra:rb, 2:W]
                )
                nc.vector.tensor_scalar_mul(ws[:, ra:rb, 0:1], tt[:, ra:rb, 1:2], 1.5)
                nc.vector.tensor_scalar_mul(
                    ws[:, ra:rb, W - 1 : W], tt[:, ra:rb, W - 1 : W], 1.5
                )

                ot = opool.tile([128, HC, W], F32)
                for p in range(NPIECE):
                    r0 = p * PIECE_ROWS
                    ps = ppool.tile([128, PIECE_ROWS, W], F32, tag="ps")
                    fix_row = None
                    if ci == 0 and p == 0:
                        fix_row = 0
                    elif ci == NCHUNK - 1 and p == NPIECE - 1:
                        fix_row = PIECE_ROWS - 1
                    if fix_row is None:
                        na, nb = 0, PIECE_ROWS
                    else:
                        na, nb = (1, PIECE_ROWS) if fix_row == 0 else (0, PIECE_ROWS - 1)
                        # boundary row: clipped H-sum + x/1.5; copy applies
                        # scale=1.5 so conv gets the row-count fix, x doesn't.
                        rr = r0 + fix_row  # chunk-local output row
                        dhs = (0, 1) if fix_row == 0 else (-1, 0)
                        for dh in dhs:
                            nc.tensor.matmul(
                                ps[:, fix_row : fix_row + 1, :],
                                wt[:],
                                ws[:, 1 + rr + dh : 2 + rr + dh, :],
                                start=(dh == dhs[0]),
                                stop=False,
                            )
                        nc.tensor.matmul(
                            ps[:, fix_row : fix_row + 1, :],
                            it15[:],
                            xt[:, 1 + rr : 2 + rr, :],
                            start=False,
                            stop=True,
                        )
                        nc.scalar.activation(
                            ot[:, rr : rr + 1, :],
                            ps[:, fix_row : fix_row + 1, :],
                            IDENT_FN,
                            bias=bt[:],
                            scale=1.5,
                        )
                    # normal rows of the piece
                    for j, dh in enumerate((-1, 0, 1)):
                        nc.tensor.matmul(
                            ps[:, na:nb, :],
                            wt[:],
                            ws[:, 1 + r0 + na + dh : 1 + r0 + nb + dh, :],
                            start=(j == 0),
                            stop=False,
                        )
                    nc.tensor.matmul(
                        ps[:, na:nb, :],
                        it[:],
                        xt[:, 1 + r0 + na : 1 + r0 + nb, :],
                        start=False,
                        stop=True,
                    )
                    nc.scalar.activation(
                        ot[:, r0 + na : r0 + nb, :],
                        ps[:, na:nb, :],
                        IDENT_FN,
                        bias=bt[:],
                    )
                nc.sync.dma_start(out=yap[p0 : p0 + 128, h0 : h0 + HC, :], in_=ot[:])
    nc.compile()
    return nc


_NC = None


def _get_nc() -> bass.Bass:
    global _NC
    if _NC is None:
        _NC = _build_nc()
    return _NC


def _host_inputs(x: np.ndarray, conv_w: np.ndarray, conv_b: np.ndarray):
    import ml_dtypes

    bf = ml_dtypes.bfloat16
    w9t = np.zeros((2 * C, 2 * C), dtype=np.float32)
    wT = (conv_w.astype(np.float32) / 9.0).T
    w9t[0:C, 0:C] = wT
    w9t[C : 2 * C, C : 2 * C] = wT
    ident = np.eye(2 * C, dtype=np.float32).astype(bf)
    ident15 = (np.eye(2 * C, dtype=np.float32) / 1.5).astype(bf)
    bias2 = np.concatenate([conv_b, conv_b]).reshape(2 * C, 1).astype(np.float32)
    x = np.ascontiguousarray(x, dtype=np.float32)
    in_maps = []
    for i in range(NCORES):
        xi = x[i * PER : (i + 1) * PER].reshape(PER * C, H, W)
        in_maps.append(
            {
                "x": xi,
                "w9t": w9t.astype(bf),
                "ident": ident,
                "ident15": ident15,
                "bias2": bias2,
            }
        )
    return in_maps


def kernel(x: np.ndarray, conv_w: np.ndarray, conv_b: np.ndarray) -> np.ndarray:
    nc = _get_nc()
    in_maps = _host_inputs(x, conv_w, conv_b)
    res = run_bass_kernel_spmd(nc, in_maps, list(range(NCORES)))
    outs = [
        np.asarray(res.results[i]["y"]).reshape(PER, C, H, W) for i in range(NCORES)
    ]
    return np.concatenate(outs, axis=0)


# revision 18
# speedup vs baseline: 1.4281x; 1.1546x over previous
"""Trainium2 Bass kernel for nn_Denoising_ResNet: out = x + conv1x1(box_mean3x3(x)) + b.

Sharding: data-parallel over batch (32 samples -> 4 per core x 8 cores).

Per-core layout: 2 "stacks" of 2 samples each -> 128 SBUF partitions
(= 2 samples x 64 channels). The image is processed in H-chunks of 32
rows with a 1-row halo.

Math decomposition (K=3 edge-clipped box mean, then 1x1 conv):
  - x is loaded as bf16 (gpsimd casting DMA).
  - W-direction 3-tap sum: two shifted DVE adds; edge cols folded to
    1.5*(2-tap).
  - Global /9 of the box mean is folded into the conv weight.
  - H-direction 3-tap sum is FUSED into the 1x1 conv: 3 accumulating
    PE matmuls with row-shifted moving operands against a block-diagonal
    [128,128] weight kron(I2, (W/9)^T) in bf16.
  - The residual +x rides the same PSUM group as a 4th accumulating
    identity matmul (kron(I2, I) bf16). Matmuls are emitted grouped by
    stationary weight to minimize LDWEIGHTS swaps.
  - PSUM -> SBUF copy + bias live on the scalar engine
    (activation Identity, bias per partition), one copy per 2-bank
    PSUM tile (8 rows).
  - Image-boundary rows use their own PSUM group: 2 clipped H-matmuls +
    identity/1.5 for x, then the scalar-engine copy applies scale=1.5
    (the edge-clip row count fix) so only the conv term is scaled 1.5x
    while x comes out unscaled.
"""

from contextlib import ExitStack

import numpy as np

import concourse.bass as bass
import concourse.tile as tile
from concourse import bacc, mybir
from concourse.bass_utils import run_bass_kernel_spmd

B, C, H, W = 32, 64, 128, 128
NCORES = 8
PER = B // NCORES  # samples per core
NSTACK = PER // 2  # 2-sample stacks per core
HC = 32  # chunk height (output rows per chunk)
NCHUNK = H // HC
GROUP_ROWS = 4  # rows per matmul accumulation group (512 f32 = 1 bank)
TILE_ROWS = 8  # rows per PSUM tile (2 banks), 2 groups per tile
NTILE = HC // TILE_ROWS

F32 = mybir.dt.float32
BF16 = mybir.dt.bfloat16


def _build_nc() -> bass.Bass:
    nc = bacc.Bacc("TRN2", debug=False)
    x = nc.dram_tensor("x", [PER * C, H, W], F32, kind="ExternalInput")
    w9t = nc.dram_tensor("w9t", [2 * C, 2 * C], BF16, kind="ExternalInput")
    ident = nc.dram_tensor("ident", [2 * C, 2 * C], BF16, kind="ExternalInput")
    w05t = nc.dram_tensor("w05t", [2 * C, 2 * C], BF16, kind="ExternalInput")
    bias2 = nc.dram_tensor("bias2", [2 * C, 1], F32, kind="ExternalInput")
    y = nc.dram_tensor("y", [PER * C, H, W], F32, kind="ExternalOutput")
    xap = x.ap()
    yap = y.ap()
    IDENT_FN = mybir.ActivationFunctionType.Identity

    with ExitStack() as ctx:
        tc = ctx.enter_context(tile.TileContext(nc))
        cpool = ctx.enter_context(tc.tile_pool(name="const", bufs=1))
        wt = cpool.tile([128, 128], BF16)
        nc.sync.dma_start(out=wt[:], in_=w9t.ap()[:, :])
        it = cpool.tile([128, 128], BF16)
        nc.sync.dma_start(out=it[:], in_=ident.ap()[:, :])
        w05 = cpool.tile([128, 128], BF16)
        nc.sync.dma_start(out=w05[:], in_=w05t.ap()[:, :])
        bt = cpool.tile([128, 1], F32)
        nc.sync.dma_start(out=bt[:], in_=bias2.ap()[:, :])

        ppool = ctx.enter_context(tc.tile_pool(name="psum", bufs=4, space="PSUM"))

        xpool = ctx.enter_context(tc.tile_pool(name="xin", bufs=4))
        tpool = ctx.enter_context(tc.tile_pool(name="tmp", bufs=2))
        wpool = ctx.enter_context(tc.tile_pool(name="wsum", bufs=2))
        opool = ctx.enter_context(tc.tile_pool(name="out", bufs=4))

        for g in range(NSTACK):
            p0 = g * 128
            for ci in range(NCHUNK):
                h0 = ci * HC
                # xt row r holds image row h0-1+r (bf16). Rows [ra, rb) are
                # real image data; out-of-image halo rows are never read.
                xt = xpool.tile([128, HC + 2, W], BF16)
                ra, rb = 0, HC + 2
                if ci == 0:
                    ra = 1
                    nc.gpsimd.dma_start(
                        out=xt[:, 1 : HC + 2, :], in_=xap[p0 : p0 + 128, 0 : HC + 1, :]
                    )
                elif ci == NCHUNK - 1:
                    rb = HC + 1
                    nc.gpsimd.dma_start(
                        out=xt[:, 0 : HC + 1, :], in_=xap[p0 : p0 + 128, h0 - 1 : H, :]
                    )
                else:
                    nc.gpsimd.dma_start(
                        out=xt[:], in_=xap[p0 : p0 + 128, h0 - 1 : h0 + HC + 1, :]
                    )

                # W-direction 3-tap on DVE (bf16).
                tt = tpool.tile([128, HC + 2, W], BF16)
                ws = wpool.tile([128, HC + 2, W], BF16)
                if ci == 0:
                    nc.vector.memset(ws[:, 0:1, :], 0.0)
                elif ci == NCHUNK - 1:
                    nc.vector.memset(ws[:, HC + 1 : HC + 2, :], 0.0)
                nc.vector.tensor_add(
                    tt[:, ra:rb, 1:W], xt[:, ra:rb, 0 : W - 1], xt[:, ra:rb, 1:W]
                )
                nc.vector.tensor_add(
                    ws[:, ra:rb, 1 : W - 1], tt[:, ra:rb, 1 : W - 1], xt[:, ra:rb, 2:W]
                )
                nc.vector.tensor_scalar_mul(ws[:, ra:rb, 0:1], tt[:, ra:rb, 1:2], 1.5)
                nc.vector.tensor_scalar_mul(
                    ws[:, ra:rb, W - 1 : W], tt[:, ra:rb, W - 1 : W], 1.5
                )

                ot = opool.tile([128, HC, W], F32)
                for tp in range(NTILE):
                    ps = ppool.tile([128, TILE_ROWS, W], F32, tag="ps")
                    t0 = tp * TILE_ROWS  # chunk-local first output row of tile
                    # fix_row: tile-local image-boundary row (row-count fix)
                    fix_row = None
                    if ci == 0 and tp == 0:
                        fix_row = 0
                    elif ci == NCHUNK - 1 and tp == NTILE - 1:
                        fix_row = TILE_ROWS - 1
                    # one accumulation group per 4-row half (= one 2KB bank):
                    # 3 H-matmuls (zero ws halo rows make the boundary rows
                    # come out clipped), then for the boundary row 2 extra
                    # 0.5x-weight matmuls (-> 1.5x conv total), then the
                    # identity matmul (+x) closes each group.
                    for hp in range(2):
                        ga, gb = hp * GROUP_ROWS, (hp + 1) * GROUP_ROWS
                        for j, dh in enumerate((-1, 0, 1)):
                            nc.tensor.matmul(
                                ps[:, ga:gb, :],
                                wt[:],
                                ws[:, 1 + t0 + ga + dh : 1 + t0 + gb + dh, :],
                                start=(j == 0),
                                stop=False,
                            )
                    if fix_row is not None:
                        for dh in ((0, 1) if fix_row == 0 else (-1, 0)):
                            nc.tensor.matmul(
                                ps[:, fix_row : fix_row + 1, :],
                                w05[:],
                                ws[:, 1 + t0 + fix_row + dh : 2 + t0 + fix_row + dh, :],
                                start=False,
                                stop=False,
                            )
                    for hp in range(2):
                        ga, gb = hp * GROUP_ROWS, (hp + 1) * GROUP_ROWS
                        nc.tensor.matmul(
                            ps[:, ga:gb, :],
                            it[:],
                            xt[:, 1 + t0 + ga : 1 + t0 + gb, :],
                            start=False,
                            stop=True,
                        )
                    nc.scalar.activation(
                        ot[:, t0 : t0 + TILE_ROWS, :],
                        ps[:],
                        IDENT_FN,
                        bias=bt[:],
                    )
                nc.sync.dma_start(out=yap[p0 : p0 + 128, h0 : h0 + HC, :], in_=ot[:])
    nc.compile()
    return nc


_NC = None


def _get_nc() -> bass.Bass:
    global _NC
    if _NC is None:
        _NC = _build_nc()
    return _NC


def _host_inputs(x: np.ndarray, conv_w: np.ndarray, conv_b: np.ndarray):
    import ml_dtypes

    bf = ml_dtypes.bfloat16
    w9t = np.zeros((2 * C, 2 * C), dtype=np.float32)
    wT = (conv_w.astype(np.float32) / 9.0).T
    w9t[0:C, 0:C] = wT
    w9t[C : 2 * C, C : 2 * C] = wT
    ident = np.eye(2 * C, dtype=np.float32).astype(bf)
    w05t = (w9t * 0.5).astype(bf)
    bias2 = np.concatenate([conv_b, conv_b]).reshape(2 * C, 1).astype(np.float32)
    x = np.ascontiguousarray(x, dtype=np.float32)
    in_maps = []
    for i in range(NCORES):
        xi = x[i * PER : (i + 1) * PER].reshape(PER * C, H, W)
        in_maps.append(
            {
                "x": xi,
                "w9t": w9t.astype(bf),
                "ident": ident,
                "w05t": w05t,
                "bias2": bias2,
            }
        )
    return in_maps


def kernel(x: np.ndarray, conv_w: np.ndarray, conv_b: np.ndarray) -> np.ndarray:
    nc = _get_nc()
    in_maps = _host_inputs(x, conv_w, conv_b)
    res = run_bass_kernel_spmd(nc, in_maps, list(range(NCORES)))
    outs = [
        np.asarray(res.results[i]["y"]).reshape(PER, C, H, W) for i in range(NCORES)
    ]
    return np.concatenate(outs, axis=0)
